# revision 70
# baseline (speedup 1.0000x reference)
"""Trainium2 Bass kernel for nn_LinformerProjectionEntireOutImg.

Math: the reference's softmax is over a constant tensor -> uniform 1/64, so
the whole net collapses to a linear pipeline:
  T[b,q,i,j]  = sum_p cp[b, p*128+q, i, :] @ wc[p*128+q, :, j]   (q = n mod 128)
  S[b, r]     = T.reshape(B, 8192),  r = q*64 + i*8 + j
  P2[b,e]     = S @ E_proj.reshape(8192, 256)
  v[b,k]      = (P2[b,k]+P2[b,64+k]+P2[b,128+k]+P2[b,192+k])/64 + rel[k]
  out[b,o,i,j]= sum_m v[b, i*8+m] * w_next[o, m, j]
Sharding: core c owns capsule groups q in [16c, 16c+16) (== heads 4c..4c+4),
batch unsharded. Each core reads a disjoint 1/8 of current_pose/w_current and
1/8 of E_proj. The pipeline is linear, so each core emits its partial output
(core 0 carries the rel_embedd affine term) and the unshard is a sum.

Precision/layout plan (HBM traffic is the bottleneck; ~180 GB/s per DGE
queue, two queues): stage 1 operands in fp8e4 (A raw randn; W pre-scaled x64
on host so its 0.02*randn values sit in e4m3's normal range), DoubleRow
matmuls (two p per instruction) split into two 64x64 PE tiles — the q-halves'
64-wide block-diagonal W shares pack columns on disjoint partition ranges,
halving the zero padding (DoubleRow requires the PSUM dst at partition 0, so
each half accumulates its own base-0 PSUM tile). The 4-way hid fold
(256->64), the softmax 1/64 and the 1/64 W-scale compensation are folded
into E on the host -> [64,1024] bf16 per half-pair, shipped on the gpsimd
software DGE so the two hardware queues carry only the A/W stream.
rel/ident/w_next ride one byte-packed [32,1344] region. Stage 2/3 run in
bf16, output in bf16 (host sums cores in fp32). Measured end-to-end rel err
~3e-3 vs the 2e-2 gate.
"""

import os

import numpy as np

_STATE: dict = {}

B, OUT_N, POSE = 32, 64, 64
NCORES = 8

# p-chunk boundaries for the streamed stage-1 operand. A chunk costs
# max(bytes/~190GB/s, 128 packets/~75us^-1) of queue time (packets never
# span partition rows), so chunks below 8p (2560B rows) are packet-limited
# and waste queue slots: six chunks of 8-12p, none thin, byte- and
# slot-balanced across the two queues. The 8p first chunk arrives no later
# than a 4p one would (same packet count) but carries twice the work; the
# 8p last chunk keeps the post-wall PE tail short. All chunks even-sized
# (DoubleRow consumes p in pairs).
P_BOUNDS = [0, 8, 20, 32, 44, 56, 64]
PRM_BYTES = 1344  # [32,x]: wn bf16 1024B | rel f32 256B | ident bf16 64B
P_UNIT = 320  # per-p pack: 256 B of A + 64 B of W (64-wide block-diag)
E_COLS = 2048  # folded-E bytes per partition ([64,2048])


def _build_nc():
    import concourse.mybir as mybir
    from concourse import bacc
    from concourse.tile import TileContext

    f32 = mybir.dt.float32
    bf16 = mybir.dt.bfloat16
    f8 = mybir.dt.float8e4
    dr = mybir.MatmulPerfMode.DoubleRow
    nc = bacc.Bacc()
    AW = nc.dram_tensor("aw_pack", [128, 64 * P_UNIT], f8, kind="ExternalInput")
    EP = nc.dram_tensor("e_pack", [64, E_COLS], f8, kind="ExternalInput")
    PRM = nc.dram_tensor("prm", [32, PRM_BYTES], f8, kind="ExternalInput")
    OUT = nc.dram_tensor("out", [128, 1024], bf16, kind="ExternalOutput")

    nchunk = len(P_BOUNDS) - 1
    with TileContext(nc) as tc:
        with (
            tc.tile_pool(name="apool", bufs=nchunk + 1) as apool,
            tc.tile_pool(name="cpool", bufs=1) as cpool,
            tc.tile_pool(name="spool", bufs=1) as spool,
            tc.tile_pool(name="pp", bufs=1, space="PSUM") as pp,
            tc.tile_pool(name="pp3", bufs=2, space="PSUM") as pp3,
        ):
            # folded E on the gpsimd software DGE: keeps the hw queues pure
            # A/W; only needed once the first half drains.
            ept = cpool.tile([64, E_COLS], f8, tag="ep")
            nc.gpsimd.dma_start(out=ept[:], in_=EP[:])
            ets = [
                ept[:, 0:1024].bitcast(bf16),
                ept[:, 1024:2048].bitcast(bf16),
            ]

            # AW chunk DMAs, alternating between the two HWDGE queues.
            awts = []
            for ci in range(nchunk):
                npair = (P_BOUNDS[ci + 1] - P_BOUNDS[ci]) // 2
                lo, hi = P_BOUNDS[ci] * P_UNIT, P_BOUNDS[ci + 1] * P_UNIT
                # scalar's queue arms ~0.6-1.3us after sync's: give sync the
                # odd chunks, including the final one that gates stage-1's
                # end; chunk 0's later arrival on scalar is absorbed by the
                # DMA-paced pipeline.
                awt = apool.tile([128, npair, 2, P_UNIT], f8, tag="aw")
                eng = (nc.scalar, nc.sync)[ci % 2]
                eng.dma_start(out=awt[:], in_=AW[:, lo:hi])
                awts.append(awt)
            # byte-packed params (w_next / rel / ident) on the scalar queue:
            # if the scheduler hoists them to the queue head, they only
            # delay chunk 0 (absorbable), not the end-gating final chunk.
            prio = tc.cur_priority
            tc.cur_priority = 500
            prmt = cpool.tile([32, PRM_BYTES], f8, tag="prm")
            nc.scalar.dma_start(out=prmt[:], in_=PRM[:])
            tc.cur_priority = prio
            wnt = prmt[0:8, 0:1024].bitcast(bf16)
            relt = prmt[:, 1024:1280].bitcast(f32)
            idt = prmt[:, 1280:1344].bitcast(bf16)

            # stage 1: T[(q,j),(i,b)] = sum_p Wblk_p.T @ A_p (64-wide
            # block-diag), two p per DoubleRow matmul, two 64x64 PE tiles per
            # pair. Four sequential quarter-chains per half (same-bank
            # accumulation pipelines fine): each quarter's PSUM drain + its
            # stage-2 matmuls hide under the next (DMA-paced) quarter; only
            # the last quarter's drain is on the tail.
            chains = [
                [
                    pp.tile([64, 256], f32, tag=f"o{g}_{k}", name=f"o{g}_{k}")
                    for k in range(2)
                ]
                for g in range(2)
            ]
            p2_ps = pp.tile([32, 64], f32, tag="p2_ps")

            def drain_quarter(qt):
                # PSUM -> SBUF (bf16) per half (vector || scalar), then 16
                # stage-2 matmuls: v[b,k] += O_qt_g[:, i-cols].T @ Ef_g_i
                # (accumulate over i, halves and quarters). Ef has the 4-way
                # hid fold, the softmax 1/64 and the W x64 compensation baked
                # in.
                osbs = []
                for g in range(2):
                    osb = spool.tile(
                        [64, 256], bf16, tag=f"osb{g}_{qt}", name=f"osb{g}_{qt}"
                    )
                    if g == 0:
                        nc.vector.tensor_copy(osb[:], chains[g][qt][:])
                    else:
                        nc.scalar.copy(osb[:], chains[g][qt][:])
                    osbs.append(osb)
                for i in range(8):
                    for g in range(2):
                        nc.tensor.matmul(
                            p2_ps[:],
                            osbs[g][:, i * 32 : (i + 1) * 32],
                            ets[g][:, i * 64 : (i + 1) * 64],
                            start=(qt == 0 and i == 0 and g == 0),
                            stop=(qt == 1 and i == 7 and g == 1),
                        )

            bounds = P_BOUNDS
            pr_base = 0
            for ci in range(nchunk):
                pv = awts[ci][:]
                for t in range((bounds[ci + 1] - bounds[ci]) // 2):
                    pr = pr_base + t
                    for g in range(2):
                        rows = slice(64 * g, 64 * (g + 1))
                        nc.tensor.matmul(
                            chains[g][pr // 16][:],
                            pv[rows, t, :, 256:320],
                            pv[rows, t, :, 0:256],
                            start=(pr % 16 == 0),
                            stop=(pr % 16 == 15),
                            perf_mode=dr,
                            tile_position=(64 * g, 0),
                        )
                    if pr % 16 == 15:
                        drain_quarter(pr // 16)
                pr_base += (bounds[ci + 1] - bounds[ci]) // 2

            # add rel (zeros on cores 1..7), cast to bf16
            vs = spool.tile([32, 64], bf16, tag="vs")
            nc.vector.tensor_add(vs[:], p2_ps[:], relt[:])

            # transpose v slices: vt[m, i*32+b] = v[b, i*8+m] (partition base 0)
            vt_ps = pp.tile([8, 256], bf16, tag="vt_ps")
            for i in range(8):
                nc.tensor.transpose(
                    vt_ps[:, i * 32 : (i + 1) * 32],
                    vs[:, i * 8 : (i + 1) * 8],
                    idt[:],
                )
            vt_sb = spool.tile([8, 256], bf16, tag="vt")
            nc.scalar.copy(vt_sb[:], vt_ps[:])

            # stage 3: out_h[(i4,b),(o,j)] = vt[:, h-cols].T @ wn[m,(o,j)]
            # Both halves land in one [128,1024] tile (2KB rows) and go out
            # as two partition-split DMAs so both queues push fat packets.
            o3_sb = spool.tile([128, 1024], bf16, tag="o3sb")
            for h in range(2):
                o3 = pp3.tile([128, 512], f32, tag="o3")
                nc.tensor.matmul(
                    o3[:],
                    vt_sb[:, h * 128 : (h + 1) * 128],
                    wnt[:],
                    start=True,
                    stop=True,
                )
                if h == 0:
                    nc.vector.tensor_copy(o3_sb[:, 0:512], o3[:])
                else:
                    nc.scalar.copy(o3_sb[:, 512:1024], o3[:])
            nc.sync.dma_start(out=OUT[0:64, :], in_=o3_sb[0:64, :])
            nc.scalar.dma_start(out=OUT[64:128, :], in_=o3_sb[64:128, :])
    nc.finalize()
    return nc


def _prepack(current_pose, w_current, w_next, E_proj, rel_embedd):
    import ml_dtypes

    f8 = ml_dtypes.float8_e4m3fn
    bf = ml_dtypes.bfloat16
    cp = np.ascontiguousarray(current_pose, dtype=np.float32)
    wc = np.ascontiguousarray(w_current, dtype=np.float32).reshape(64, 8, 16, 8, 8)
    # A_all[c, p, (q,m), (i,b)]
    cp6 = cp.reshape(B, 64, 8, 16, 8, 8)  # (b, p, c, q, i, m)
    a_all = np.ascontiguousarray(cp6.transpose(2, 1, 3, 5, 4, 0), dtype=f8).reshape(
        8, 64, 128, 256
    )
    # W[c, p, (q,m), (q mod 8, j)]: 64-wide block-diagonal, the two q-halves
    # sharing columns on disjoint partition ranges; x64 so 0.02*randn values
    # land in e4m3's normal range (compensated in E)
    w_all = np.zeros((8, 64, 16, 8, 8, 8), dtype=f8)
    wc_t = np.ascontiguousarray(wc.transpose(1, 0, 2, 3, 4)) * 64.0  # (c,p,q,m,j)
    wc_t8 = wc_t.astype(f8)
    for q in range(16):
        w_all[:, :, q, :, q % 8, :] = wc_t8[:, :, q]
    w_all = w_all.reshape(8, 64, 128, 64)
    aw_all = np.concatenate([a_all, w_all], axis=-1)  # (c, p, 128, 320)
    aw_all = np.ascontiguousarray(aw_all.transpose(0, 2, 1, 3)).reshape(
        8, 128, 64 * P_UNIT
    )
    # Ef[c, (q,j), (i,k)]: hid fold (sum over h4), /64 softmax mean, /64
    # W-scale; split by q-half, both halves at partitions 0-63
    e6 = (np.asarray(E_proj, dtype=np.float32) / 4096.0).reshape(
        8, 4, 4, 8, 8, 4, 64
    )  # (c, qh, q4, i, j, h4, k)
    ef = e6.sum(axis=5)  # (c, qh, q4, i, j, k)
    ef = np.ascontiguousarray(ef.transpose(0, 1, 2, 4, 3, 5), dtype=bf).reshape(
        8, 128, 512
    )  # (c, (qh,q4,j), (i,k))
    e_pack = (
        np.concatenate([ef[:, 0:64], ef[:, 64:128]], axis=2)
        .view(np.uint8)
        .view(f8)
    )  # (c, 64, 2048 bytes)

    # byte-packed params [32, 1344]: wn bf16 | rel f32 | ident bf16
    wn_pack = np.zeros((32, 512), dtype=bf)
    wn_pack[0:8] = np.asarray(w_next, dtype=np.float32).transpose(1, 0, 2).reshape(
        8, 512
    )
    prm = np.zeros((8, 32, PRM_BYTES), dtype=np.uint8)
    prm[:, :, 0:1024] = wn_pack.view(np.uint8)[None]
    # rel: only core 0 carries the affine term
    rel0 = np.broadcast_to(
        np.asarray(rel_embedd, dtype=np.float32).reshape(1, 64), (32, 64)
    )
    prm[0, :, 1024:1280] = np.ascontiguousarray(rel0).view(np.uint8)
    prm[:, :, 1280:1344] = np.eye(32, dtype=bf).view(np.uint8)[None]
    prm = prm.view(f8)

    return [
        {"aw_pack": aw_all[c], "e_pack": e_pack[c], "prm": prm[c]}
        for c in range(NCORES)
    ]


def kernel(current_pose, w_current, w_next, E_proj, rel_embedd):
    from concourse import bass_utils

    if "nc" not in _STATE:
        _STATE["nc"] = _build_nc()
    nc = _STATE["nc"]
    in_maps = _prepack(current_pose, w_current, w_next, E_proj, rel_embedd)
    trace = os.environ.get("KERNEL_TRACE") == "1"
    res = bass_utils.run_bass_kernel_spmd(
        nc, in_maps, core_ids=list(range(NCORES)), trace=trace
    )
    _STATE["last_result"] = res
    acc = np.zeros((128, 1024), dtype=np.float32)
    for c in range(NCORES):
        acc += np.asarray(res.results[c]["out"], dtype=np.float32)
    # [(i4, b), (h, o, j)] -> (b, o, h*4+i4, j)
    out = (
        acc.reshape(4, 32, 2, 64, 8)
        .transpose(1, 3, 2, 0, 4)
        .reshape(B, OUT_N, POSE)
    )
    return np.ascontiguousarray(out[:, None, :, :])


# revision 71
# speedup vs baseline: 1.0600x; 1.0600x over previous
"""Trainium2 Bass kernel for nn_LinformerProjectionEntireOutImg.

Math: the reference's softmax is over a constant tensor -> uniform 1/64, so
the whole net collapses to a linear pipeline:
  T[b,q,i,j]  = sum_p cp[b, p*128+q, i, :] @ wc[p*128+q, :, j]   (q = n mod 128)
  S[b, r]     = T.reshape(B, 8192),  r = q*64 + i*8 + j
  P2[b,e]     = S @ E_proj.reshape(8192, 256)
  v[b,k]      = (P2[b,k]+P2[b,64+k]+P2[b,128+k]+P2[b,192+k])/64 + rel[k]
  out[b,o,i,j]= sum_m v[b, i*8+m] * w_next[o, m, j]
Sharding: core c owns capsule groups q in [16c, 16c+16) (== heads 4c..4c+4),
batch unsharded. Each core reads a disjoint 1/8 of current_pose/w_current and
1/8 of E_proj. The pipeline is linear, so each core emits its partial output
(core 0 carries the rel_embedd affine term) and the unshard is a sum.

Precision/layout plan (HBM traffic is the bottleneck; ~180 GB/s per DGE
queue, two queues): stage 1 operands in fp8e4 (A raw randn; W pre-scaled x64
on host so its 0.02*randn values sit in e4m3's normal range), DoubleRow
matmuls (two p per instruction) split into two 64x64 PE tiles — the q-halves'
64-wide block-diagonal W shares pack columns on disjoint partition ranges,
halving the zero padding (DoubleRow requires the PSUM dst at partition 0, so
each half accumulates its own base-0 PSUM tile). The 4-way hid fold
(256->64), the softmax 1/64 and the 1/64 W-scale compensation are folded
into E on the host -> [64,1024] bf16 per half-pair, shipped on the gpsimd
software DGE so the two hardware queues carry only the A/W stream.
rel/ident/w_next ride one byte-packed [32,1344] region. Stage 2/3 run in
bf16, output in bf16 (host sums cores in fp32). Measured end-to-end rel err
~3e-3 vs the 2e-2 gate.
"""

import os

import numpy as np

_STATE: dict = {}

B, OUT_N, POSE = 32, 64, 64
NCORES = 8

# p-chunk boundaries for the streamed stage-1 operand. A chunk costs
# max(bytes/~190GB/s, 128 packets/~75us^-1) of queue time (packets never
# span partition rows), so chunks below 8p (2560B rows) are packet-limited
# and waste queue slots: six chunks of 8-12p, none thin, byte- and
# slot-balanced across the two queues. The 8p first chunk arrives no later
# than a 4p one would (same packet count) but carries twice the work; the
# 8p last chunk keeps the post-wall PE tail short. All chunks even-sized
# (DoubleRow consumes p in pairs).
P_BOUNDS = [0, 8, 20, 32, 44, 56, 64]
PRM_BYTES = 1344  # [32,x]: wn bf16 1024B | rel f32 256B | ident bf16 64B
P_UNIT = 320  # per-p pack: 256 B of A + 64 B of W (64-wide block-diag)
E_COLS = 2048  # folded-E bytes per partition ([64,2048])


def _build_nc():
    import concourse.mybir as mybir
    from concourse import bacc
    from concourse.tile import TileContext

    f32 = mybir.dt.float32
    bf16 = mybir.dt.bfloat16
    f8 = mybir.dt.float8e4
    dr = mybir.MatmulPerfMode.DoubleRow
    nc = bacc.Bacc()
    AW = nc.dram_tensor("aw_pack", [128, 64 * P_UNIT], f8, kind="ExternalInput")
    EP = nc.dram_tensor("e_pack", [64, E_COLS], f8, kind="ExternalInput")
    PRM = nc.dram_tensor("prm", [32, PRM_BYTES], f8, kind="ExternalInput")
    OUT = nc.dram_tensor("out", [128, 1024], bf16, kind="ExternalOutput")

    nchunk = len(P_BOUNDS) - 1
    with TileContext(nc) as tc:
        with (
            tc.tile_pool(name="apool", bufs=nchunk + 1) as apool,
            tc.tile_pool(name="cpool", bufs=1) as cpool,
            tc.tile_pool(name="spool", bufs=1) as spool,
            tc.tile_pool(name="pp", bufs=1, space="PSUM") as pp,
            tc.tile_pool(name="pp3", bufs=2, space="PSUM") as pp3,
        ):
            # folded E on the gpsimd software DGE: keeps the hw queues pure
            # A/W; only needed once the first half drains.
            ept = cpool.tile([64, E_COLS], f8, tag="ep")
            nc.gpsimd.dma_start(out=ept[:], in_=EP[:])
            ets = [
                ept[:, 0:1024].bitcast(bf16),
                ept[:, 1024:2048].bitcast(bf16),
            ]

            # AW chunk DMAs, alternating between the two HWDGE queues.
            awts = []
            for ci in range(nchunk):
                npair = (P_BOUNDS[ci + 1] - P_BOUNDS[ci]) // 2
                lo, hi = P_BOUNDS[ci] * P_UNIT, P_BOUNDS[ci + 1] * P_UNIT
                awt = apool.tile([128, npair, 2, P_UNIT], f8, tag="aw")
                eng = (nc.sync, nc.scalar)[ci % 2]
                eng.dma_start(out=awt[:], in_=AW[:, lo:hi])
                awts.append(awt)
            # byte-packed params (w_next / rel / ident), deprioritized so the
            # scheduler keeps them behind the A stream on the sync queue.
            prio = tc.cur_priority
            tc.cur_priority = 500
            prmt = cpool.tile([32, PRM_BYTES], f8, tag="prm")
            nc.sync.dma_start(out=prmt[:], in_=PRM[:])
            tc.cur_priority = prio
            wnt = prmt[0:8, 0:1024].bitcast(bf16)
            relt = prmt[:, 1024:1280].bitcast(f32)
            idt = prmt[:, 1280:1344].bitcast(bf16)

            # stage 1: T[(q,j),(i,b)] = sum_p Wblk_p.T @ A_p (64-wide
            # block-diag), two p per DoubleRow matmul, two 64x64 PE tiles per
            # pair. Four sequential quarter-chains per half (same-bank
            # accumulation pipelines fine): each quarter's PSUM drain + its
            # stage-2 matmuls hide under the next (DMA-paced) quarter; only
            # the last quarter's drain is on the tail.
            chains = [
                [
                    pp.tile([64, 256], f32, tag=f"o{g}_{k}", name=f"o{g}_{k}")
                    for k in range(2)
                ]
                for g in range(2)
            ]
            p2_ps = pp.tile([32, 64], f32, tag="p2_ps")

            def drain_quarter(qt):
                # PSUM -> SBUF (bf16) per half (vector || scalar), then 16
                # stage-2 matmuls: v[b,k] += O_qt_g[:, i-cols].T @ Ef_g_i
                # (accumulate over i, halves and quarters). Ef has the 4-way
                # hid fold, the softmax 1/64 and the W x64 compensation baked
                # in.
                osbs = []
                for g in range(2):
                    osb = spool.tile(
                        [64, 256], bf16, tag=f"osb{g}_{qt}", name=f"osb{g}_{qt}"
                    )
                    if g == 0:
                        nc.vector.tensor_copy(osb[:], chains[g][qt][:])
                    else:
                        nc.scalar.copy(osb[:], chains[g][qt][:])
                    osbs.append(osb)
                for i in range(8):
                    for g in range(2):
                        nc.tensor.matmul(
                            p2_ps[:],
                            osbs[g][:, i * 32 : (i + 1) * 32],
                            ets[g][:, i * 64 : (i + 1) * 64],
                            start=(qt == 0 and i == 0 and g == 0),
                            stop=(qt == 1 and i == 7 and g == 1),
                        )

            bounds = P_BOUNDS
            pr_base = 0
            for ci in range(nchunk):
                pv = awts[ci][:]
                for t in range((bounds[ci + 1] - bounds[ci]) // 2):
                    pr = pr_base + t
                    for g in range(2):
                        rows = slice(64 * g, 64 * (g + 1))
                        nc.tensor.matmul(
                            chains[g][pr // 16][:],
                            pv[rows, t, :, 256:320],
                            pv[rows, t, :, 0:256],
                            start=(pr % 16 == 0),
                            stop=(pr % 16 == 15),
                            perf_mode=dr,
                            tile_position=(64 * g, 0),
                        )
                    if pr % 16 == 15:
                        drain_quarter(pr // 16)
                pr_base += (bounds[ci + 1] - bounds[ci]) // 2

            # add rel (zeros on cores 1..7), cast to bf16
            vs = spool.tile([32, 64], bf16, tag="vs")
            nc.vector.tensor_add(vs[:], p2_ps[:], relt[:])

            # transpose v slices: vt[m, i*32+b] = v[b, i*8+m] (partition base 0)
            vt_ps = pp.tile([8, 256], bf16, tag="vt_ps")
            for i in range(8):
                nc.tensor.transpose(
                    vt_ps[:, i * 32 : (i + 1) * 32],
                    vs[:, i * 8 : (i + 1) * 8],
                    idt[:],
                )
            vt_sb = spool.tile([8, 256], bf16, tag="vt")
            nc.scalar.copy(vt_sb[:], vt_ps[:])

            # stage 3: out_h[(i4,b),(o,j)] = vt[:, h-cols].T @ wn[m,(o,j)]
            # Both halves land in one [128,1024] tile (2KB rows) and go out
            # as two partition-split DMAs so both queues push fat packets.
            o3_sb = spool.tile([128, 1024], bf16, tag="o3sb")
            for h in range(2):
                o3 = pp3.tile([128, 512], f32, tag="o3")
                nc.tensor.matmul(
                    o3[:],
                    vt_sb[:, h * 128 : (h + 1) * 128],
                    wnt[:],
                    start=True,
                    stop=True,
                )
                if h == 0:
                    nc.vector.tensor_copy(o3_sb[:, 0:512], o3[:])
                else:
                    nc.scalar.copy(o3_sb[:, 512:1024], o3[:])
            nc.sync.dma_start(out=OUT[0:64, :], in_=o3_sb[0:64, :])
            nc.scalar.dma_start(out=OUT[64:128, :], in_=o3_sb[64:128, :])
    nc.finalize()
    return nc


def _prepack(current_pose, w_current, w_next, E_proj, rel_embedd):
    import ml_dtypes

    f8 = ml_dtypes.float8_e4m3fn
    bf = ml_dtypes.bfloat16
    cp = np.ascontiguousarray(current_pose, dtype=np.float32)
    wc = np.ascontiguousarray(w_current, dtype=np.float32).reshape(64, 8, 16, 8, 8)
    # A_all[c, p, (q,m), (i,b)]
    cp6 = cp.reshape(B, 64, 8, 16, 8, 8)  # (b, p, c, q, i, m)
    a_all = np.ascontiguousarray(cp6.transpose(2, 1, 3, 5, 4, 0), dtype=f8).reshape(
        8, 64, 128, 256
    )
    # W[c, p, (q,m), (q mod 8, j)]: 64-wide block-diagonal, the two q-halves
    # sharing columns on disjoint partition ranges; x64 so 0.02*randn values
    # land in e4m3's normal range (compensated in E)
    w_all = np.zeros((8, 64, 16, 8, 8, 8), dtype=f8)
    wc_t = np.ascontiguousarray(wc.transpose(1, 0, 2, 3, 4)) * 64.0  # (c,p,q,m,j)
    wc_t8 = wc_t.astype(f8)
    for q in range(16):
        w_all[:, :, q, :, q % 8, :] = wc_t8[:, :, q]
    w_all = w_all.reshape(8, 64, 128, 64)
    aw_all = np.concatenate([a_all, w_all], axis=-1)  # (c, p, 128, 320)
    aw_all = np.ascontiguousarray(aw_all.transpose(0, 2, 1, 3)).reshape(
        8, 128, 64 * P_UNIT
    )
    # Ef[c, (q,j), (i,k)]: hid fold (sum over h4), /64 softmax mean, /64
    # W-scale; split by q-half, both halves at partitions 0-63
    e6 = (np.asarray(E_proj, dtype=np.float32) / 4096.0).reshape(
        8, 4, 4, 8, 8, 4, 64
    )  # (c, qh, q4, i, j, h4, k)
    ef = e6.sum(axis=5)  # (c, qh, q4, i, j, k)
    ef = np.ascontiguousarray(ef.transpose(0, 1, 2, 4, 3, 5), dtype=bf).reshape(
        8, 128, 512
    )  # (c, (qh,q4,j), (i,k))
    e_pack = (
        np.concatenate([ef[:, 0:64], ef[:, 64:128]], axis=2)
        .view(np.uint8)
        .view(f8)
    )  # (c, 64, 2048 bytes)

    # byte-packed params [32, 1344]: wn bf16 | rel f32 | ident bf16
    wn_pack = np.zeros((32, 512), dtype=bf)
    wn_pack[0:8] = np.asarray(w_next, dtype=np.float32).transpose(1, 0, 2).reshape(
        8, 512
    )
    prm = np.zeros((8, 32, PRM_BYTES), dtype=np.uint8)
    prm[:, :, 0:1024] = wn_pack.view(np.uint8)[None]
    # rel: only core 0 carries the affine term
    rel0 = np.broadcast_to(
        np.asarray(rel_embedd, dtype=np.float32).reshape(1, 64), (32, 64)
    )
    prm[0, :, 1024:1280] = np.ascontiguousarray(rel0).view(np.uint8)
    prm[:, :, 1280:1344] = np.eye(32, dtype=bf).view(np.uint8)[None]
    prm = prm.view(f8)

    return [
        {"aw_pack": aw_all[c], "e_pack": e_pack[c], "prm": prm[c]}
        for c in range(NCORES)
    ]


def kernel(current_pose, w_current, w_next, E_proj, rel_embedd):
    from concourse import bass_utils

    if "nc" not in _STATE:
        _STATE["nc"] = _build_nc()
    nc = _STATE["nc"]
    in_maps = _prepack(current_pose, w_current, w_next, E_proj, rel_embedd)
    trace = os.environ.get("KERNEL_TRACE") == "1"
    res = bass_utils.run_bass_kernel_spmd(
        nc, in_maps, core_ids=list(range(NCORES)), trace=trace
    )
    _STATE["last_result"] = res
    acc = np.zeros((128, 1024), dtype=np.float32)
    for c in range(NCORES):
        acc += np.asarray(res.results[c]["out"], dtype=np.float32)
    # [(i4, b), (h, o, j)] -> (b, o, h*4+i4, j)
    out = (
        acc.reshape(4, 32, 2, 64, 8)
        .transpose(1, 3, 2, 0, 4)
        .reshape(B, OUT_N, POSE)
    )
    return np.ascontiguousarray(out[:, None, :, :])
